# revision 6
# baseline (speedup 1.0000x reference)
"""Trainium2 Bass kernel for nn_Attention_9887014715893.

Multi-head attention forward (B=1, S=4096, D=1024, H=16, E=64, fp32):
    qkv = x @ w_qkv ; q,k,v per head ; softmax(q k^T / 8 + mask) @ v

Sharding: tensor-parallel over heads. 8 cores x 2 heads each. Each core gets
the full x (transposed + bf16-cast on host) and its own 128-column bf16
slices of w_qkv, and produces out[:, 128c:128c+128]. No collectives.

v2 (bf16): fp32 matmuls run the PE at the 1.2 GHz "others" clock; bf16
operands run it at 2.4 GHz and halve SBUF/DMA footprints. All matmul
operands (x, w, q, k, v, exp scores) are bf16; PSUM accumulation stays
fp32, as do the attention accumulators and the output. Host-simulated
max-abs rel err of the full bf16 pipeline: ~6e-3 (gate 2e-2).

Per-core algorithm:
  - proj: QT2/KT2 [128, 4096] bf16 (two heads stacked on partitions,
    1/sqrt(E) folded into wq on host). V projected directly in transposed
    layout (out[s,e] from lhsT=x-tile) into va [128, 65*32] bf16 with a
    ones column so the softmax denominator falls out of the attn@V matmul.
  - attention, scores TRANSPOSED (k on partitions, q on free axis):
      scT[128, 512] = KT-slice^T x QT-slice   (PE, fp32 PSUM)
      exT = exp(scT)                          (ACT, PSUM->SBUF, bf16)
      accT[65, 512] += va[kt]^T @ exT         (PE, fp32 PSUM accum)
    Projection of x s-chunks is interleaved with early attention units so
    the ACT engine (the bottleneck: 33.5M exps/core) starts ~10us in.
  - epilogue: DMA raw [65, q] accumulators to HBM; divide by the
    denominator row + final [e,s]->[s,e] transpose happen on the host.
"""

import sys

if "/opt/trn_rl_repo" not in sys.path:
    sys.path.insert(0, "/opt/trn_rl_repo")

import numpy as np
from contextlib import ExitStack

import concourse.bass as bass
import concourse.bacc as bacc
import concourse.tile as tile
import concourse.mybir as mybir
from concourse.bass_utils import run_bass_kernel_spmd

F32 = mybir.dt.float32
BF16 = mybir.dt.bfloat16
EXP = mybir.ActivationFunctionType.Exp

S = 4096          # sequence length
DM = 1024         # model dim
E = 64            # head dim
NCORES = 8
EC = 128          # output columns per core (2 heads x 64)
QC = 512          # q chunk (free axis of transposed scores)
NQ = S // QC      # 8 q chunks
NK = S // 128     # 32 k tiles
ND = DM // 128    # 8 d tiles


def _build_kernel(with_mask: bool):
    nc = bacc.Bacc("TRN2", target_bir_lowering=False, debug=False,
                   enable_asserts=False, num_devices=NCORES)
    xT = nc.dram_tensor("xT", [DM, S], BF16, kind="ExternalInput").ap()
    wq = nc.dram_tensor("wq", [DM, EC], BF16, kind="ExternalInput").ap()
    wk = nc.dram_tensor("wk", [DM, EC], BF16, kind="ExternalInput").ap()
    wv = nc.dram_tensor("wv", [DM, EC], BF16, kind="ExternalInput").ap()
    if with_mask:
        maskT = nc.dram_tensor("maskT", [S, S], BF16, kind="ExternalInput").ap()
    # raw transposed output: rows 0-64 head0 {outT | denom}, 65-129 head1.
    outT = nc.dram_tensor("outT", [130, S], F32, kind="ExternalOutput").ap()

    with tile.TileContext(nc) as tc, ExitStack() as ctx:
        w_pool = ctx.enter_context(tc.tile_pool(name="w", bufs=1))
        wq_sb = w_pool.tile([128, DM], BF16)
        wk_sb = w_pool.tile([128, DM], BF16)
        wv_sb = w_pool.tile([128, DM], BF16)
        for t in range(ND):
            nc.sync.dma_start(wq_sb[:, 128 * t:128 * (t + 1)], wq[128 * t:128 * (t + 1), :])
            nc.sync.dma_start(wk_sb[:, 128 * t:128 * (t + 1)], wk[128 * t:128 * (t + 1), :])
            nc.sync.dma_start(wv_sb[:, 128 * t:128 * (t + 1)], wv[128 * t:128 * (t + 1), :])

        qt_pool = ctx.enter_context(tc.tile_pool(name="qt", bufs=1))
        QT2 = qt_pool.tile([128, S], BF16)   # rows 0-63 head0 e-dims, 64-127 head1
        KT2 = qt_pool.tile([128, S], BF16)
        va_pool = ctx.enter_context(tc.tile_pool(name="va", bufs=1))
        va = [va_pool.tile([128, 65 * NK], BF16, name=f"va{h}") for h in range(2)]
        ones_b = va_pool.tile([128, 1], BF16)
        nc.vector.memset(ones_b[:], 1.0)
        for h in range(2):
            nc.vector.tensor_copy(va[h][:, 64:65 * NK:65],
                                  ones_b[:].to_broadcast([128, NK]))

        # full x kept resident in SBUF (8 chunks x [128, 8*512] bf16 = 64KB/par)
        xs_pool = ctx.enter_context(tc.tile_pool(name="xs", bufs=1))
        xs = [xs_pool.tile([128, ND * QC], BF16, name=f"xs{c}") for c in range(NQ)]

        # PSUM: psA 3 slots x 2 banks (scores + proj psums), psB 2 x 1 bank
        psA = ctx.enter_context(tc.tile_pool(name="psA", bufs=3, space="PSUM"))
        psB = ctx.enter_context(tc.tile_pool(name="psB", bufs=2, space="PSUM"))

        exp_pool = ctx.enter_context(tc.tile_pool(name="exp", bufs=8))
        accsb_pool = ctx.enter_context(tc.tile_pool(name="accsb", bufs=4))
        if with_mask:
            msk_pool = ctx.enter_context(tc.tile_pool(name="msk", bufs=3))

        def dma_chunk(c):
            s0 = QC * c
            for t in range(ND):
                nc.sync.dma_start(xs[c][:, QC * t:QC * (t + 1)],
                                  xT[128 * t:128 * (t + 1), s0:s0 + QC])

        def proj_qk(wsb, dst, c, ps=None, t_range=None):
            # t_range lets the 8 accumulation matmuls be split across call
            # sites (shorter PE bursts -> ACT never starves); pass the same
            # psum tile to the continuation call.
            s0 = QC * c
            if ps is None:
                ps = psA.tile([128, QC], F32, tag="psA")
            ts = range(ND) if t_range is None else t_range
            for t in ts:
                nc.tensor.matmul(ps[:], lhsT=wsb[:, 128 * t:128 * (t + 1)],
                                 rhs=xs[c][:, QC * t:QC * (t + 1)],
                                 start=(t == 0), stop=(t == ND - 1))
            if t_range is None or ts[-1] == ND - 1:
                nc.vector.tensor_copy(dst[:, s0:s0 + QC], ps[:])
            return ps

        def proj_v(c):
            # direct transposed V: out[s-tile, e] = sum_t x-tile^T @ wv-tile
            for st in range(4):
                kk = 4 * c + st
                ps = psA.tile([128, 128], F32, tag="psA")
                for t in range(ND):
                    nc.tensor.matmul(
                        ps[:],
                        lhsT=xs[c][:, QC * t + 128 * st:QC * t + 128 * (st + 1)],
                        rhs=wv_sb[:, 128 * t:128 * (t + 1)],
                        start=(t == 0), stop=(t == ND - 1))
                nc.vector.tensor_copy(va[0][:, 65 * kk:65 * kk + 64], ps[:, 0:64])
                nc.vector.tensor_copy(va[1][:, 65 * kk:65 * kk + 64], ps[:, 64:128])

        # ---- attention unit: one pair of k-tiles for (qc, h) ----
        def attn_unit(qc, h, kp, accs):
            q0 = QC * qc
            k0 = 256 * kp
            if with_mask:
                msk = msk_pool.tile([128, 1024], BF16, tag="msk")
                nc.sync.dma_start(msk[:, 0:512], maskT[k0:k0 + 128, q0:q0 + 512])
                nc.sync.dma_start(msk[:, 512:1024],
                                  maskT[k0 + 128:k0 + 256, q0:q0 + 512])
            sc = psA.tile([128, 1024], F32, tag="psA", name=f"sc{qc}_{h}_{kp}")
            for c2 in range(2):
                nc.tensor.matmul(
                    sc[:, 512 * c2:512 * (c2 + 1)],
                    lhsT=KT2[64 * h:64 * (h + 1), k0 + 128 * c2:k0 + 128 * (c2 + 1)],
                    rhs=QT2[64 * h:64 * (h + 1), q0:q0 + QC],
                    start=True, stop=True,
                    tile_position=(64 * h, 0),
                )
            if with_mask:
                nc.vector.tensor_tensor(out=sc[:], in0=sc[:], in1=msk[:],
                                        op=mybir.AluOpType.add)
            ex = exp_pool.tile([128, 1024], BF16, tag="exp", name=f"ex{qc}_{h}_{kp}")
            nc.scalar.activation(ex[:], sc[:], EXP)
            for c2 in range(2):
                kk = 2 * kp + c2
                nc.tensor.matmul(
                    accs[:],
                    lhsT=va[h][:, 65 * kk:65 * kk + 65],
                    rhs=ex[:, 512 * c2:512 * (c2 + 1)],
                    start=(kk == 0), stop=(kk == NK - 1),
                )

        def epilogue(qc, h, accs):
            asb = accsb_pool.tile([65, QC], F32, tag="accsb")
            nc.vector.tensor_copy(asb[:], accs[:])
            nc.sync.dma_start(outT[65 * h:65 * h + 65, QC * qc:QC * (qc + 1)], asb[:])

        # ---------------- emission: proj with early attention interleave ----
        # Both q0 heads' attention units are interleaved into the projection
        # phase as their k-tiles become ready, so the ACT engine (the
        # bottleneck) starts ~15us in and never starves thereafter.
        for c in range(3):
            dma_chunk(c)
        accs0 = [psB.tile([65, QC], F32, tag="psB", name=f"acc0_{h}")
                 for h in range(2)]
        next_kp = [0, 0]

        def emit_q0_units(h0_max, h1_max):
            # alternate heads, h1 lagging h0 by one unit
            h0_max = min(h0_max, NK // 2 - 1)
            h1_max = min(h1_max, NK // 2 - 1)
            while next_kp[0] <= h0_max or next_kp[1] <= h1_max:
                if next_kp[0] <= h0_max:
                    attn_unit(0, 0, next_kp[0], accs0[0])
                    next_kp[0] += 1
                if next_kp[1] <= h1_max:
                    attn_unit(0, 1, next_kp[1], accs0[1])
                    next_kp[1] += 1

        for c in range(NQ):
            if c + 3 < NQ:
                dma_chunk(c + 3)
            proj_qk(wk_sb, KT2, c)
            proj_v(c)
            if c == 0:
                proj_qk(wq_sb, QT2, 0)
            emit_q0_units(2 * c + 1, 2 * c)
        emit_q0_units(NK // 2 - 1, NK // 2 - 1)
        epilogue(0, 0, accs0[0])
        epilogue(0, 1, accs0[1])
        # Q proj for chunk 1 (was not interleaved during the proj phase)
        proj_qk(wq_sb, QT2, 1)

        # remaining (qc, h) blocks; Q proj for qc+1 interleaved into the h1
        # block in two 4-matmul bursts so ACT's ~2-unit buffer survives.
        for qc in range(1, NQ):
            for h in range(2):
                accs = psB.tile([65, QC], F32, tag="psB", name=f"acc{qc}_{h}")
                qps = None
                for kp in range(NK // 2):
                    attn_unit(qc, h, kp, accs)
                    if h == 1 and qc < NQ - 1:
                        if kp == 2:
                            qps = proj_qk(wq_sb, QT2, qc + 1, t_range=range(0, 4))
                        elif kp == 8:
                            proj_qk(wq_sb, QT2, qc + 1, ps=qps,
                                    t_range=range(4, 8))
                epilogue(qc, h, accs)

    nc.compile()
    return nc


_CACHE: dict = {}


def _get_kernel(with_mask: bool):
    if with_mask not in _CACHE:
        _CACHE[with_mask] = _build_kernel(with_mask)
    return _CACHE[with_mask]


def _bf16(a):
    import ml_dtypes
    return np.ascontiguousarray(a).astype(ml_dtypes.bfloat16)


def build_in_maps(x, w_qkv, maskT=None):
    xTb = _bf16(x[0].T)                                    # [DM, S]
    scale = np.float32(1.0 / np.sqrt(E))
    in_maps = []
    for c in range(NCORES):
        m = {
            "xT": xTb,
            "wq": _bf16(w_qkv[:, EC * c:EC * (c + 1)] * scale),
            "wk": _bf16(w_qkv[:, DM + EC * c:DM + EC * (c + 1)]),
            "wv": _bf16(w_qkv[:, 2 * DM + EC * c:2 * DM + EC * (c + 1)]),
        }
        if maskT is not None:
            m["maskT"] = maskT
        in_maps.append(m)
    return in_maps


def kernel(x: np.ndarray, mask: np.ndarray, w_qkv: np.ndarray) -> np.ndarray:
    x = np.asarray(x, dtype=np.float32)
    mask = np.asarray(mask, dtype=np.float32)
    w_qkv = np.asarray(w_qkv, dtype=np.float32)
    assert x.shape == (1, S, DM) and w_qkv.shape == (DM, 3 * DM)

    with_mask = bool(np.any(mask))
    nc = _get_kernel(with_mask)

    maskT = None
    if with_mask:
        maskT = _bf16(np.broadcast_to(mask, (1, 1, S, S))[0, 0].T)
    in_maps = build_in_maps(x, w_qkv, maskT)

    res = run_bass_kernel_spmd(nc, in_maps, core_ids=list(range(NCORES)))
    # host-side normalize (softmax denominator is row 64/129) and transpose
    outs = []
    for c in range(NCORES):
        o = res.results[c]["outT"]                       # [130, S]
        h0 = o[0:64] / o[64:65]
        h1 = o[65:129] / o[129:130]
        outs.append(np.concatenate([h0, h1], axis=0).T)  # [S, 128]
    return np.ascontiguousarray(
        np.concatenate(outs, axis=1), dtype=np.float32).reshape(1, S, DM)
